# revision 23
# baseline (speedup 1.0000x reference)
"""GATv2 attention-weights kernel for 8 Trainium2 NeuronCores.

Problem (per full input):
    q: (2, 8, 384, 64) f32, k: (2, 8, 384, 64) f32,
    attention: (1, 8, 1, 1, 64) f32, mask: (2, 8, 384, 384) bool
    scores[b,h,i,j] = sum_d silu(q[b,h,i,d] + k[b,h,j,d]) * attention[h,d]
    out = softmax over j with mask (-inf before, 0 after)

Sharding: data-parallel over the 16 (b,h) pairs, 2 per core.

Per-core device pipeline (raw bass, explicit semaphores; "jj,d" packing =
two j columns share the 128 partitions, d=64 each half):
    - DVE builds T[(jj,d), i] = qT_rep + k_pair  (per-partition-scalar add,
      2x perf mode) for G j-pairs per group
    - ACT computes silu IN PLACE on T (ACT is the throughput floor:
      LQ*LK*D silu evaluations per (b,h) at 128 lanes / 1.2 GHz)
    - PE reduces over d with the `a` vector folded into the weights:
      matmul(lhsT=T_block[(jj,d), i_block], rhs=a2[(jj,d), 2]) ->
      scores[i_block, j_pair] land un-transposed in PSUM (6 banks hold all
      scores for both (b,h))
    - Masked softmax over the free dim afterwards (one activation-table
      switch to Exp for the whole kernel): fused (mask*-1e30)+scores on DVE,
      exp with fused row-sum (accum_out) on ACT, reciprocal + scale on DVE.
      No per-row max: scores are bounded (|s| < 8), exp cannot overflow.
"""

import numpy as np
from contextlib import ExitStack

import concourse.bass as bass
from concourse import mybir
from concourse.bass_utils import run_bass_kernel_spmd

B, H, LQ, LK, D = 2, 8, 384, 384, 64
NCORES = 8
NBH = (B * H) // NCORES        # 2 (b,h) pairs per core
NPAIR = LK // 2                # 192 j-pairs
# j-pairs per silu group: bh0 ramps up so the pipeline fills fast, then
# steady-state groups are large to amortize the ACT per-instruction overhead
GROUPS_BH = [[2, 4, 8, 12, 16, 20, 26, 32, 32, 32, 8], [32, 32, 32, 32, 32, 24, 8]]
assert all(sum(g) == NPAIR for g in GROUPS_BH)
GMAX = max(max(g) for g in GROUPS_BH)
# flattened per-rep schedule: (bh, size, pair_offset)
GLIST = [(bh, s, off)
         for bh in range(2)
         for s, off in zip(GROUPS_BH[bh],
                           [sum(GROUPS_BH[bh][:i])
                            for i in range(len(GROUPS_BH[bh]))])]
G0 = len(GROUPS_BH[0])         # groups in bh0
GG = len(GLIST)                # global groups per rep
NIB = LQ // 128                # 3 i-blocks
NSM = NBH * NIB                # 6 softmax tiles
QKA = LQ + NPAIR + 2           # packed constants width (per partition, f32)

_f32 = mybir.dt.float32
_u8 = mybir.dt.uint8

_built = None  # cache across calls


def _build(reps=1):
    # reps > 1 unrolls the whole computation N times inside one program
    # (used only for steady-state timing; the grading path uses reps=1).
    AF = mybir.ActivationFunctionType
    Alu = mybir.AluOpType

    nc = bass.Bass("TRN2", target_bir_lowering=False, debug=False,
                   num_devices=NCORES)

    qka_d = nc.dram_tensor("qka", [NBH, 128, QKA], _f32, kind="ExternalInput").ap()
    mask_d = nc.dram_tensor("masku8", [NBH, LQ, LK], _u8, kind="ExternalInput").ap()
    w_d = nc.dram_tensor("w", [NBH, LQ, LK], _f32, kind="ExternalOutput").ap()

    qka_t = [nc.alloc_sbuf_tensor(f"qka_t{bh}", [128, QKA], _f32).ap()
             for bh in range(NBH)]
    mask_t = [nc.alloc_sbuf_tensor(f"mask_t{i}", [128, LK], _u8).ap()
              for i in range(NSM)]
    T_t = [nc.alloc_sbuf_tensor(f"T{s}", [128, GMAX * LQ], _f32).ap()
           for s in range(3)]
    scm_t = [nc.alloc_sbuf_tensor(f"scm{i}", [128, LK], _f32).ap()
             for i in range(NSM)]
    E_t = [nc.alloc_sbuf_tensor(f"E{i}", [128, LK], _f32).ap()
           for i in range(NSM)]
    W_t = [nc.alloc_sbuf_tensor(f"W{i}", [128, LK], _f32).ap()
           for i in range(NSM)]
    sums_t = [nc.alloc_sbuf_tensor(f"sums{i}", [128, 1], _f32).ap()
              for i in range(NSM)]
    r_t = [nc.alloc_sbuf_tensor(f"r{i}", [128, 1], _f32).ap()
           for i in range(NSM)]
    sc_t = [nc.alloc_psum_tensor(f"sc{i}", [128, LK], _f32).ap()
            for i in range(NSM)]

    def qtrep(bh):
        return qka_t[bh][:, 0:LQ]

    def kpair(bh, p):
        return qka_t[bh][:, LQ + p:LQ + p + 1]

    def a2(bh):
        return qka_t[bh][:, LQ + NPAIR:LQ + NPAIR + 2]

    with ExitStack() as ctx:
        s_qka = [ctx.enter_context(nc.semaphore(f"s_qka{bh}")) for bh in range(NBH)]
        s_qk0b = ctx.enter_context(nc.semaphore("s_qk0b"))
        s_qk0c = ctx.enter_context(nc.semaphore("s_qk0c"))
        s_mask = ctx.enter_context(nc.semaphore("s_mask"))
        s_T = ctx.enter_context(nc.semaphore("s_T"))
        s_S = ctx.enter_context(nc.semaphore("s_S"))
        s_pe = ctx.enter_context(nc.semaphore("s_pe"))
        s_scm = ctx.enter_context(nc.semaphore("s_scm"))
        s_E = ctx.enter_context(nc.semaphore("s_E"))
        s_W = ctx.enter_context(nc.semaphore("s_W"))
        s_wsp = ctx.enter_context(nc.semaphore("s_wsp"))
        s_wact = ctx.enter_context(nc.semaphore("s_wact"))
        block = ctx.enter_context(nc.Block())

        CH0 = LQ + 16  # first chunk: qtrep + 16 kpairs (gates group 0..2)

        @block.sync
        def _(sp):
            # bh0 leading chunk first (everything upstream gates on it),
            # a2 rides in the same chunk via a second tiny DMA on the queue
            sp.dma_start(out=qka_t[0][:, 0:CH0],
                         in_=qka_d[0][:, 0:CH0]).then_inc(s_qka[0], 16)
            sp.dma_start(out=qka_t[0][:, LQ + NPAIR:LQ + NPAIR + 2],
                         in_=qka_d[0][:, LQ + NPAIR:LQ + NPAIR + 2]
                         ).then_inc(s_qk0b, 16)
            sp.dma_start(out=qka_t[0][:, CH0:LQ + NPAIR],
                         in_=qka_d[0][:, CH0:LQ + NPAIR]
                         ).then_inc(s_qk0c, 16)
            sp.dma_start(out=qka_t[1], in_=qka_d[1]).then_inc(s_qka[1], 16)
            for idx in range(NSM):
                bh, ib = divmod(idx, NIB)
                sp.dma_start(out=mask_t[idx],
                             in_=mask_d[bh, ib * 128:(ib + 1) * 128, :]
                             ).then_inc(s_mask, 16)
            # output DMAs: late tiles (3,4,5) on the SP HWDGE queue,
            # early tiles (0,1,2) on ACT's HWDGE queue (see scalar block)
            for rep in range(reps):
                for idx in range(NSM // 2, NSM):
                    bh, ib = divmod(idx, NIB)
                    sp.wait_ge(s_W, rep * NSM + idx + 1)
                    sp.dma_start(out=w_d[bh, ib * 128:(ib + 1) * 128, :],
                                 in_=W_t[idx]).then_inc(s_wsp, 16)
            sp.wait_ge(s_wsp, 16 * (NSM // 2) * reps)
            sp.wait_ge(s_wact, 16 * (NSM // 2) * reps)

        @block.vector
        def _(v):
            def tbuild(v, rep, gg):
                bh, size, off = GLIST[gg]
                gi = rep * GG + gg
                if rep == 0 and bh == 1 and off == 0:
                    v.wait_ge(s_qka[1], 16)
                if gi >= 3:
                    v.wait_ge(s_pe, gi - 2)
                T = T_t[gi % 3]
                for pl in range(size):
                    p = off + pl
                    ins = v.tensor_scalar_add(
                        T[:, pl * LQ:(pl + 1) * LQ], qtrep(bh), kpair(bh, p))
                ins.then_inc(s_T, 1)

            def scm(v, rep, bh):
                # mask+scores fuse for this bh.  bh0's is emitted a few
                # groups into bh1's stream so its s_pe wait is already
                # implied and DVE never stalls at the bh boundary.
                if rep == 0 and bh == 0:
                    v.wait_ge(s_mask, 16 * NSM)
                for ib in range(NIB):
                    idx = bh * NIB + ib
                    v.wait_ge(s_pe, rep * GG + (G0 if bh == 0 else GG))
                    if rep >= 1:
                        # scm tile reuse: previous rep's exp must be done
                        v.wait_ge(s_E, (rep - 1) * NSM + idx + 1)
                    v.scalar_tensor_tensor(
                        scm_t[idx], mask_t[idx], -1e30, sc_t[idx],
                        Alu.mult, Alu.add).then_inc(s_scm, 1)

            v.wait_ge(s_qka[0], 16)
            for rep in range(reps):
                for gg in range(GG):
                    bh_, size_, off_ = GLIST[gg]
                    if rep == 0 and bh_ == 0 and off_ < 16 <= off_ + size_:
                        v.wait_ge(s_qk0c, 16)
                    tbuild(v, rep, gg)
                    if gg == G0 + 2:
                        scm(v, rep, 0)  # bh0 softmax prep, overlapped
                scm(v, rep, 1)
                for idx in range(NSM):
                    v.wait_ge(s_E, rep * NSM + idx + 1)
                    if rep >= 1:
                        # W tile reuse: all of the previous rep's output DMAs
                        # on the owning queue must be done (conservative --
                        # cross-DMA order within a queue isn't assumed)
                        qs = s_wact if idx < NSM // 2 else s_wsp
                        v.wait_ge(qs, 16 * (NSM // 2) * rep)
                    v.reciprocal(r_t[idx], sums_t[idx])
                    v.drain()  # r is a scalar operand of the next op
                    v.tensor_scalar_mul(W_t[idx], E_t[idx],
                                        r_t[idx]).then_inc(s_W, 1)

        @block.scalar
        def _(a):
            for rep in range(reps):
                for gg in range(GG):
                    _, size, _ = GLIST[gg]
                    a.wait_ge(s_T, rep * GG + gg + 1)
                    T = T_t[(rep * GG + gg) % 3]
                    a.activation(T[:, 0:size * LQ], T[:, 0:size * LQ],
                                 AF.Silu).then_inc(s_S, 1)
                for idx in range(NSM):
                    a.wait_ge(s_scm, rep * NSM + idx + 1)
                    if rep >= 1:
                        # E/sums tile reuse: previous rep's W-scale must be done
                        a.wait_ge(s_W, (rep - 1) * NSM + idx + 1)
                    a.activation(E_t[idx], scm_t[idx], AF.Exp,
                                 accum_out=sums_t[idx]).then_inc(s_E, 1)
                # early output tiles on ACT's HWDGE queue (waits are
                # already satisfied by the time the last exp retires)
                for idx in range(NSM // 2):
                    bh, ib = divmod(idx, NIB)
                    a.wait_ge(s_W, rep * NSM + idx + 1)
                    a.dma_start(out=w_d[bh, ib * 128:(ib + 1) * 128, :],
                                in_=W_t[idx]).then_inc(s_wact, 16)


        @block.tensor
        def _(t):
            t.wait_ge(s_qk0b, 16)  # a2 rides in its own tiny chunk
            for rep in range(reps):
                for gg in range(GG):
                    bh, size, off = GLIST[gg]
                    if rep == 0 and bh == 1 and off == 0:
                        t.wait_ge(s_qka[1], 16)
                    if rep >= 1 and off == 0:
                        # sc bank reuse: previous rep's scm reads must be done
                        t.wait_ge(s_scm, (rep - 1) * NSM + NIB * (bh + 1))
                    t.wait_ge(s_S, rep * GG + gg + 1)
                    T = T_t[(rep * GG + gg) % 3]
                    for pl in range(size):
                        p = off + pl
                        for ib in range(NIB):
                            ins = nc.tensor.matmul(
                                sc_t[bh * NIB + ib][:, 2 * p:2 * p + 2],
                                T[:, pl * LQ + ib * 128: pl * LQ + (ib + 1) * 128],
                                a2(bh),
                                start=True, stop=True)
                    ins.then_inc(s_pe, 1)

    return nc


def _shard(q, k, a, mask):
    qf = q.reshape(B * H, LQ, D)
    kf = k.reshape(B * H, LK, D)
    mf = mask.reshape(B * H, LQ, LK)
    af = np.ascontiguousarray(
        np.broadcast_to(a.reshape(1, H, D), (B, H, D))).reshape(B * H, D)
    in_maps = []
    for c in range(NCORES):
        sl = slice(NBH * c, NBH * (c + 1))
        qT = qf[sl].transpose(0, 2, 1)                                # [NBH,64,LQ]
        kp = kf[sl].reshape(NBH, NPAIR, 2, D).transpose(0, 2, 3, 1)   # [NBH,2,D,NPAIR]
        qka = np.zeros((NBH, 128, QKA), np.float32)
        qka[:, 0:64, 0:LQ] = qT
        qka[:, 64:128, 0:LQ] = qT
        qka[:, :, LQ:LQ + NPAIR] = kp.reshape(NBH, 128, NPAIR)
        for j in range(NBH):
            qka[j, 0:64, LQ + NPAIR] = af[NBH * c + j]
            qka[j, 64:128, LQ + NPAIR + 1] = af[NBH * c + j]
        masku8 = np.ascontiguousarray(mf[sl]).astype(np.uint8)
        in_maps.append(dict(qka=qka, masku8=masku8))
    return in_maps


def kernel(q, k, attention, mask):
    global _built
    q = np.asarray(q, np.float32)
    k = np.asarray(k, np.float32)
    a = np.asarray(attention, np.float32)
    mask = np.asarray(mask).astype(bool)

    in_maps = _shard(q, k, a, mask)
    if _built is None:
        _built = _build()
    res = run_bass_kernel_spmd(_built, in_maps, core_ids=list(range(NCORES)))
    w = np.stack([res.results[c]["w"] for c in range(NCORES)], axis=0)
    return w.reshape(B, H, LQ, LK).astype(np.float32)


# revision 24
# speedup vs baseline: 1.0041x; 1.0041x over previous
"""GATv2 attention-weights kernel for 8 Trainium2 NeuronCores.

Problem (per full input):
    q: (2, 8, 384, 64) f32, k: (2, 8, 384, 64) f32,
    attention: (1, 8, 1, 1, 64) f32, mask: (2, 8, 384, 384) bool
    scores[b,h,i,j] = sum_d silu(q[b,h,i,d] + k[b,h,j,d]) * attention[h,d]
    out = softmax over j with mask (-inf before, 0 after)

Sharding: data-parallel over the 16 (b,h) pairs, 2 per core.

Per-core device pipeline (raw bass, explicit semaphores; "jj,d" packing =
two j columns share the 128 partitions, d=64 each half):
    - DVE builds T[(jj,d), i] = qT_rep + k_pair  (per-partition-scalar add,
      2x perf mode) for G j-pairs per group
    - ACT computes silu IN PLACE on T (ACT is the throughput floor:
      LQ*LK*D silu evaluations per (b,h) at 128 lanes / 1.2 GHz)
    - PE reduces over d with the `a` vector folded into the weights:
      matmul(lhsT=T_block[(jj,d), i_block], rhs=a2[(jj,d), 2]) ->
      scores[i_block, j_pair] land un-transposed in PSUM (6 banks hold all
      scores for both (b,h))
    - Masked softmax over the free dim afterwards (one activation-table
      switch to Exp for the whole kernel): fused (mask*-1e30)+scores on DVE,
      exp with fused row-sum (accum_out) on ACT, reciprocal + scale on DVE.
      No per-row max: scores are bounded (|s| < 8), exp cannot overflow.
"""

import numpy as np
from contextlib import ExitStack

import concourse.bass as bass
from concourse import mybir
from concourse.bass_utils import run_bass_kernel_spmd

B, H, LQ, LK, D = 2, 8, 384, 384, 64
NCORES = 8
NBH = (B * H) // NCORES        # 2 (b,h) pairs per core
NPAIR = LK // 2                # 192 j-pairs
# j-pairs per silu group: bh0 ramps up so the pipeline fills fast, then
# steady-state groups are large to amortize the ACT per-instruction overhead
GROUPS_BH = [[2, 4, 8, 8, 12, 16, 20, 26, 32, 32, 32], [32, 32, 32, 32, 32, 24, 8]]
assert all(sum(g) == NPAIR for g in GROUPS_BH)
GMAX = max(max(g) for g in GROUPS_BH)
# flattened per-rep schedule: (bh, size, pair_offset)
GLIST = [(bh, s, off)
         for bh in range(2)
         for s, off in zip(GROUPS_BH[bh],
                           [sum(GROUPS_BH[bh][:i])
                            for i in range(len(GROUPS_BH[bh]))])]
G0 = len(GROUPS_BH[0])         # groups in bh0
GG = len(GLIST)                # global groups per rep
NIB = LQ // 128                # 3 i-blocks
NSM = NBH * NIB                # 6 softmax tiles
QKA = LQ + NPAIR + 2           # packed constants width (per partition, f32)

_f32 = mybir.dt.float32
_u8 = mybir.dt.uint8

_built = None  # cache across calls


def _build(reps=1):
    # reps > 1 unrolls the whole computation N times inside one program
    # (used only for steady-state timing; the grading path uses reps=1).
    AF = mybir.ActivationFunctionType
    Alu = mybir.AluOpType

    nc = bass.Bass("TRN2", target_bir_lowering=False, debug=False,
                   num_devices=NCORES)

    qka_d = nc.dram_tensor("qka", [NBH, 128, QKA], _f32, kind="ExternalInput").ap()
    mask_d = nc.dram_tensor("masku8", [NBH, LQ, LK], _u8, kind="ExternalInput").ap()
    w_d = nc.dram_tensor("w", [NBH, LQ, LK], _f32, kind="ExternalOutput").ap()

    qka_t = [nc.alloc_sbuf_tensor(f"qka_t{bh}", [128, QKA], _f32).ap()
             for bh in range(NBH)]
    mask_t = [nc.alloc_sbuf_tensor(f"mask_t{i}", [128, LK], _u8).ap()
              for i in range(NSM)]
    T_t = [nc.alloc_sbuf_tensor(f"T{s}", [128, GMAX * LQ], _f32).ap()
           for s in range(3)]
    scm_t = [nc.alloc_sbuf_tensor(f"scm{i}", [128, LK], _f32).ap()
             for i in range(NSM)]
    E_t = [nc.alloc_sbuf_tensor(f"E{i}", [128, LK], _f32).ap()
           for i in range(NSM)]
    W_t = [nc.alloc_sbuf_tensor(f"W{i}", [128, LK], _f32).ap()
           for i in range(NSM)]
    sums_t = [nc.alloc_sbuf_tensor(f"sums{i}", [128, 1], _f32).ap()
              for i in range(NSM)]
    r_t = [nc.alloc_sbuf_tensor(f"r{i}", [128, 1], _f32).ap()
           for i in range(NSM)]
    sc_t = [nc.alloc_psum_tensor(f"sc{i}", [128, LK], _f32).ap()
            for i in range(NSM)]

    def qtrep(bh):
        return qka_t[bh][:, 0:LQ]

    def kpair(bh, p):
        return qka_t[bh][:, LQ + p:LQ + p + 1]

    def a2(bh):
        return qka_t[bh][:, LQ + NPAIR:LQ + NPAIR + 2]

    with ExitStack() as ctx:
        s_qka = [ctx.enter_context(nc.semaphore(f"s_qka{bh}")) for bh in range(NBH)]
        s_qk0b = ctx.enter_context(nc.semaphore("s_qk0b"))
        s_qk0c = ctx.enter_context(nc.semaphore("s_qk0c"))
        s_mask = ctx.enter_context(nc.semaphore("s_mask"))
        s_T = ctx.enter_context(nc.semaphore("s_T"))
        s_S = ctx.enter_context(nc.semaphore("s_S"))
        s_pe = ctx.enter_context(nc.semaphore("s_pe"))
        s_scm = ctx.enter_context(nc.semaphore("s_scm"))
        s_E = ctx.enter_context(nc.semaphore("s_E"))
        s_W = ctx.enter_context(nc.semaphore("s_W"))
        s_wsp = ctx.enter_context(nc.semaphore("s_wsp"))
        s_wact = ctx.enter_context(nc.semaphore("s_wact"))
        block = ctx.enter_context(nc.Block())

        CH0 = LQ + 16  # first chunk: qtrep + 16 kpairs (gates group 0..2)

        @block.sync
        def _(sp):
            # bh0 leading chunk first (everything upstream gates on it),
            # a2 rides in the same chunk via a second tiny DMA on the queue
            sp.dma_start(out=qka_t[0][:, 0:CH0],
                         in_=qka_d[0][:, 0:CH0]).then_inc(s_qka[0], 16)
            sp.dma_start(out=qka_t[0][:, LQ + NPAIR:LQ + NPAIR + 2],
                         in_=qka_d[0][:, LQ + NPAIR:LQ + NPAIR + 2]
                         ).then_inc(s_qk0b, 16)
            sp.dma_start(out=qka_t[0][:, CH0:LQ + NPAIR],
                         in_=qka_d[0][:, CH0:LQ + NPAIR]
                         ).then_inc(s_qk0c, 16)
            sp.dma_start(out=qka_t[1], in_=qka_d[1]).then_inc(s_qka[1], 16)
            for idx in range(NSM):
                bh, ib = divmod(idx, NIB)
                sp.dma_start(out=mask_t[idx],
                             in_=mask_d[bh, ib * 128:(ib + 1) * 128, :]
                             ).then_inc(s_mask, 16)
            # output DMAs: late tiles (3,4,5) on the SP HWDGE queue,
            # early tiles (0,1,2) on ACT's HWDGE queue (see scalar block)
            for rep in range(reps):
                for idx in range(NSM // 2, NSM):
                    bh, ib = divmod(idx, NIB)
                    sp.wait_ge(s_W, rep * NSM + idx + 1)
                    sp.dma_start(out=w_d[bh, ib * 128:(ib + 1) * 128, :],
                                 in_=W_t[idx]).then_inc(s_wsp, 16)
            sp.wait_ge(s_wsp, 16 * (NSM // 2) * reps)
            sp.wait_ge(s_wact, 16 * (NSM // 2) * reps)

        @block.vector
        def _(v):
            def tbuild(v, rep, gg):
                bh, size, off = GLIST[gg]
                gi = rep * GG + gg
                if rep == 0 and bh == 1 and off == 0:
                    v.wait_ge(s_qka[1], 16)
                if gi >= 3:
                    v.wait_ge(s_pe, gi - 2)
                T = T_t[gi % 3]
                for pl in range(size):
                    p = off + pl
                    ins = v.tensor_scalar_add(
                        T[:, pl * LQ:(pl + 1) * LQ], qtrep(bh), kpair(bh, p))
                ins.then_inc(s_T, 1)

            def scm(v, rep, bh):
                # mask+scores fuse for this bh.  bh0's is emitted a few
                # groups into bh1's stream so its s_pe wait is already
                # implied and DVE never stalls at the bh boundary.
                if rep == 0 and bh == 0:
                    v.wait_ge(s_mask, 16 * NSM)
                for ib in range(NIB):
                    idx = bh * NIB + ib
                    v.wait_ge(s_pe, rep * GG + (G0 if bh == 0 else GG))
                    if rep >= 1:
                        # scm tile reuse: previous rep's exp must be done
                        v.wait_ge(s_E, (rep - 1) * NSM + idx + 1)
                    v.scalar_tensor_tensor(
                        scm_t[idx], mask_t[idx], -1e30, sc_t[idx],
                        Alu.mult, Alu.add).then_inc(s_scm, 1)

            v.wait_ge(s_qka[0], 16)
            for rep in range(reps):
                for gg in range(GG):
                    bh_, size_, off_ = GLIST[gg]
                    if rep == 0 and bh_ == 0 and off_ < 16 <= off_ + size_:
                        v.wait_ge(s_qk0c, 16)
                    tbuild(v, rep, gg)
                    if gg == G0 + 2:
                        scm(v, rep, 0)  # bh0 softmax prep, overlapped
                scm(v, rep, 1)
                for idx in range(NSM):
                    v.wait_ge(s_E, rep * NSM + idx + 1)
                    if rep >= 1:
                        # W tile reuse: all of the previous rep's output DMAs
                        # on the owning queue must be done (conservative --
                        # cross-DMA order within a queue isn't assumed)
                        qs = s_wact if idx < NSM // 2 else s_wsp
                        v.wait_ge(qs, 16 * (NSM // 2) * rep)
                    v.reciprocal(r_t[idx], sums_t[idx])
                    v.drain()  # r is a scalar operand of the next op
                    v.tensor_scalar_mul(W_t[idx], E_t[idx],
                                        r_t[idx]).then_inc(s_W, 1)

        @block.scalar
        def _(a):
            for rep in range(reps):
                for gg in range(GG):
                    _, size, _ = GLIST[gg]
                    a.wait_ge(s_T, rep * GG + gg + 1)
                    T = T_t[(rep * GG + gg) % 3]
                    a.activation(T[:, 0:size * LQ], T[:, 0:size * LQ],
                                 AF.Silu).then_inc(s_S, 1)
                for idx in range(NSM):
                    a.wait_ge(s_scm, rep * NSM + idx + 1)
                    if rep >= 1:
                        # E/sums tile reuse: previous rep's W-scale must be done
                        a.wait_ge(s_W, (rep - 1) * NSM + idx + 1)
                    a.activation(E_t[idx], scm_t[idx], AF.Exp,
                                 accum_out=sums_t[idx]).then_inc(s_E, 1)
                # early output tiles on ACT's HWDGE queue (waits are
                # already satisfied by the time the last exp retires)
                for idx in range(NSM // 2):
                    bh, ib = divmod(idx, NIB)
                    a.wait_ge(s_W, rep * NSM + idx + 1)
                    a.dma_start(out=w_d[bh, ib * 128:(ib + 1) * 128, :],
                                in_=W_t[idx]).then_inc(s_wact, 16)


        @block.tensor
        def _(t):
            t.wait_ge(s_qk0b, 16)  # a2 rides in its own tiny chunk
            for rep in range(reps):
                for gg in range(GG):
                    bh, size, off = GLIST[gg]
                    if rep == 0 and bh == 1 and off == 0:
                        t.wait_ge(s_qka[1], 16)
                    if rep >= 1 and off == 0:
                        # sc bank reuse: previous rep's scm reads must be done
                        t.wait_ge(s_scm, (rep - 1) * NSM + NIB * (bh + 1))
                    t.wait_ge(s_S, rep * GG + gg + 1)
                    T = T_t[(rep * GG + gg) % 3]
                    for pl in range(size):
                        p = off + pl
                        for ib in range(NIB):
                            ins = nc.tensor.matmul(
                                sc_t[bh * NIB + ib][:, 2 * p:2 * p + 2],
                                T[:, pl * LQ + ib * 128: pl * LQ + (ib + 1) * 128],
                                a2(bh),
                                start=True, stop=True)
                    ins.then_inc(s_pe, 1)

    return nc


def _shard(q, k, a, mask):
    qf = q.reshape(B * H, LQ, D)
    kf = k.reshape(B * H, LK, D)
    mf = mask.reshape(B * H, LQ, LK)
    af = np.ascontiguousarray(
        np.broadcast_to(a.reshape(1, H, D), (B, H, D))).reshape(B * H, D)
    in_maps = []
    for c in range(NCORES):
        sl = slice(NBH * c, NBH * (c + 1))
        qT = qf[sl].transpose(0, 2, 1)                                # [NBH,64,LQ]
        kp = kf[sl].reshape(NBH, NPAIR, 2, D).transpose(0, 2, 3, 1)   # [NBH,2,D,NPAIR]
        qka = np.zeros((NBH, 128, QKA), np.float32)
        qka[:, 0:64, 0:LQ] = qT
        qka[:, 64:128, 0:LQ] = qT
        qka[:, :, LQ:LQ + NPAIR] = kp.reshape(NBH, 128, NPAIR)
        for j in range(NBH):
            qka[j, 0:64, LQ + NPAIR] = af[NBH * c + j]
            qka[j, 64:128, LQ + NPAIR + 1] = af[NBH * c + j]
        masku8 = np.ascontiguousarray(mf[sl]).astype(np.uint8)
        in_maps.append(dict(qka=qka, masku8=masku8))
    return in_maps


def kernel(q, k, attention, mask):
    global _built
    q = np.asarray(q, np.float32)
    k = np.asarray(k, np.float32)
    a = np.asarray(attention, np.float32)
    mask = np.asarray(mask).astype(bool)

    in_maps = _shard(q, k, a, mask)
    if _built is None:
        _built = _build()
    res = run_bass_kernel_spmd(_built, in_maps, core_ids=list(range(NCORES)))
    w = np.stack([res.results[c]["w"] for c in range(NCORES)], axis=0)
    return w.reshape(B, H, LQ, LK).astype(np.float32)


# revision 28
# speedup vs baseline: 1.0058x; 1.0017x over previous
"""GATv2 attention-weights kernel for 8 Trainium2 NeuronCores.

Problem (per full input):
    q: (2, 8, 384, 64) f32, k: (2, 8, 384, 64) f32,
    attention: (1, 8, 1, 1, 64) f32, mask: (2, 8, 384, 384) bool
    scores[b,h,i,j] = sum_d silu(q[b,h,i,d] + k[b,h,j,d]) * attention[h,d]
    out = softmax over j with mask (-inf before, 0 after)

Sharding: data-parallel over the 16 (b,h) pairs, 2 per core.

Per-core device pipeline (raw bass, explicit semaphores; "jj,d" packing =
two j columns share the 128 partitions, d=64 each half):
    - DVE builds T[(jj,d), i] = qT_rep + k_pair  (per-partition-scalar add,
      2x perf mode) for G j-pairs per group
    - ACT computes silu IN PLACE on T (ACT is the throughput floor:
      LQ*LK*D silu evaluations per (b,h) at 128 lanes / 1.2 GHz)
    - PE reduces over d with the `a` vector folded into the weights:
      matmul(lhsT=T_block[(jj,d), i_block], rhs=a2[(jj,d), 2]) ->
      scores[i_block, j_pair] land un-transposed in PSUM (6 banks hold all
      scores for both (b,h))
    - Masked softmax over the free dim afterwards (one activation-table
      switch to Exp for the whole kernel): fused (mask*-1e30)+scores on DVE,
      exp with fused row-sum (accum_out) on ACT, reciprocal + scale on DVE.
      No per-row max: scores are bounded (|s| < 8), exp cannot overflow.
"""

import numpy as np
from contextlib import ExitStack

import concourse.bass as bass
from concourse import mybir
from concourse.bass_utils import run_bass_kernel_spmd

B, H, LQ, LK, D = 2, 8, 384, 384, 64
NCORES = 8
NBH = (B * H) // NCORES        # 2 (b,h) pairs per core
NPAIR = LK // 2                # 192 j-pairs
# j-pairs per silu group: bh0 ramps up so the pipeline fills fast, then
# steady-state groups are large to amortize the ACT per-instruction overhead
GROUPS_BH = [[2, 4, 8, 8, 12, 16, 20, 26, 32, 36, 28], [36, 36, 36, 36, 36, 12]]
assert all(sum(g) == NPAIR for g in GROUPS_BH)
GMAX = max(max(g) for g in GROUPS_BH)
# flattened per-rep schedule: (bh, size, pair_offset)
GLIST = [(bh, s, off)
         for bh in range(2)
         for s, off in zip(GROUPS_BH[bh],
                           [sum(GROUPS_BH[bh][:i])
                            for i in range(len(GROUPS_BH[bh]))])]
G0 = len(GROUPS_BH[0])         # groups in bh0
GG = len(GLIST)                # global groups per rep
NIB = LQ // 128                # 3 i-blocks
NSM = NBH * NIB                # 6 softmax tiles
QKA = LQ + NPAIR + 2           # packed constants width (per partition, f32)

_f32 = mybir.dt.float32
_u8 = mybir.dt.uint8

_built = None  # cache across calls


def _build(reps=1):
    # reps > 1 unrolls the whole computation N times inside one program
    # (used only for steady-state timing; the grading path uses reps=1).
    AF = mybir.ActivationFunctionType
    Alu = mybir.AluOpType

    nc = bass.Bass("TRN2", target_bir_lowering=False, debug=False,
                   num_devices=NCORES)

    qka_d = nc.dram_tensor("qka", [NBH, 128, QKA], _f32, kind="ExternalInput").ap()
    mask_d = nc.dram_tensor("masku8", [NBH, LQ, LK], _u8, kind="ExternalInput").ap()
    w_d = nc.dram_tensor("w", [NBH, LQ, LK], _f32, kind="ExternalOutput").ap()

    qka_t = [nc.alloc_sbuf_tensor(f"qka_t{bh}", [128, QKA], _f32).ap()
             for bh in range(NBH)]
    mask_t = [nc.alloc_sbuf_tensor(f"mask_t{i}", [128, LK], _u8).ap()
              for i in range(NSM)]
    T_t = [nc.alloc_sbuf_tensor(f"T{s}", [128, GMAX * LQ], _f32).ap()
           for s in range(3)]
    E_t = [nc.alloc_sbuf_tensor(f"E{i}", [128, LK], _f32).ap()
           for i in range(NSM)]
    W_t = [nc.alloc_sbuf_tensor(f"W{i}", [128, LK], _f32).ap()
           for i in range(NSM)]
    sums_t = [nc.alloc_sbuf_tensor(f"sums{i}", [128, 1], _f32).ap()
              for i in range(NSM)]
    r_t = [nc.alloc_sbuf_tensor(f"r{i}", [128, 1], _f32).ap()
           for i in range(NSM)]
    sc_t = [nc.alloc_psum_tensor(f"sc{i}", [128, LK], _f32).ap()
            for i in range(NSM)]

    def qtrep(bh):
        return qka_t[bh][:, 0:LQ]

    def kpair(bh, p):
        return qka_t[bh][:, LQ + p:LQ + p + 1]

    def a2(bh):
        return qka_t[bh][:, LQ + NPAIR:LQ + NPAIR + 2]

    with ExitStack() as ctx:
        s_qka = [ctx.enter_context(nc.semaphore(f"s_qka{bh}")) for bh in range(NBH)]
        s_qk0b = ctx.enter_context(nc.semaphore("s_qk0b"))
        s_qk0c = ctx.enter_context(nc.semaphore("s_qk0c"))
        s_mask = ctx.enter_context(nc.semaphore("s_mask"))
        s_T = ctx.enter_context(nc.semaphore("s_T"))
        s_S = ctx.enter_context(nc.semaphore("s_S"))
        s_pe = ctx.enter_context(nc.semaphore("s_pe"))
        s_scm = ctx.enter_context(nc.semaphore("s_scm"))
        s_E = ctx.enter_context(nc.semaphore("s_E"))
        s_W = ctx.enter_context(nc.semaphore("s_W"))
        s_wsp = ctx.enter_context(nc.semaphore("s_wsp"))
        s_wact = ctx.enter_context(nc.semaphore("s_wact"))
        block = ctx.enter_context(nc.Block())

        CH0 = LQ + 16  # first chunk: qtrep + 16 kpairs (gates group 0..2)

        @block.sync
        def _(sp):
            # bh0 leading chunk first (everything upstream gates on it),
            # a2 rides in the same chunk via a second tiny DMA on the queue
            sp.dma_start(out=qka_t[0][:, 0:CH0],
                         in_=qka_d[0][:, 0:CH0]).then_inc(s_qka[0], 16)
            sp.dma_start(out=qka_t[0][:, LQ + NPAIR:LQ + NPAIR + 2],
                         in_=qka_d[0][:, LQ + NPAIR:LQ + NPAIR + 2]
                         ).then_inc(s_qk0b, 16)
            sp.dma_start(out=qka_t[0][:, CH0:LQ + NPAIR],
                         in_=qka_d[0][:, CH0:LQ + NPAIR]
                         ).then_inc(s_qk0c, 16)
            sp.dma_start(out=qka_t[1], in_=qka_d[1]).then_inc(s_qka[1], 16)
            for idx in range(NSM):
                bh, ib = divmod(idx, NIB)
                sp.dma_start(out=mask_t[idx],
                             in_=mask_d[bh, ib * 128:(ib + 1) * 128, :]
                             ).then_inc(s_mask, 16)
            # output DMAs: late tiles (3,4,5) on the SP HWDGE queue,
            # early tiles (0,1,2) on ACT's HWDGE queue (see scalar block)
            for rep in range(reps):
                for idx in range(NSM // 2, NSM):
                    bh, ib = divmod(idx, NIB)
                    sp.wait_ge(s_W, rep * NSM + idx + 1)
                    sp.dma_start(out=w_d[bh, ib * 128:(ib + 1) * 128, :],
                                 in_=W_t[idx]).then_inc(s_wsp, 16)
            sp.wait_ge(s_wsp, 16 * (NSM // 2) * reps)
            sp.wait_ge(s_wact, 16 * (NSM // 2) * reps)

        @block.vector
        def _(v):
            def tbuild(v, rep, gg):
                bh, size, off = GLIST[gg]
                gi = rep * GG + gg
                if rep == 0 and bh == 1 and off == 0:
                    v.wait_ge(s_qka[1], 16)
                if gi >= 3:
                    v.wait_ge(s_pe, gi - 2)
                T = T_t[gi % 3]
                for pl in range(size):
                    p = off + pl
                    ins = v.tensor_scalar_add(
                        T[:, pl * LQ:(pl + 1) * LQ], qtrep(bh), kpair(bh, p))
                ins.then_inc(s_T, 1)

            def scm(v, rep, bh):
                # mask+scores fuse for this bh.  bh0's is emitted a few
                # groups into bh1's stream so its s_pe wait is already
                # implied and DVE never stalls at the bh boundary.
                if rep == 0 and bh == 0:
                    v.wait_ge(s_mask, 16 * NSM)
                for ib in range(NIB):
                    idx = bh * NIB + ib
                    v.wait_ge(s_pe, rep * GG + (G0 if bh == 0 else GG))
                    if rep >= 1:
                        # scm tile reuse: previous rep's exp must be done
                        v.wait_ge(s_E, (rep - 1) * NSM + idx + 1)
                    v.scalar_tensor_tensor(
                        sc_t[idx], mask_t[idx], -1e30, sc_t[idx],
                        Alu.mult, Alu.add).then_inc(s_scm, 1)

            v.wait_ge(s_qka[0], 16)
            for rep in range(reps):
                for gg in range(GG):
                    bh_, size_, off_ = GLIST[gg]
                    if rep == 0 and bh_ == 0 and off_ < 16 <= off_ + size_:
                        v.wait_ge(s_qk0c, 16)
                    tbuild(v, rep, gg)
                    if gg == G0 + 2:
                        scm(v, rep, 0)  # bh0 softmax prep, overlapped
                scm(v, rep, 1)
                for idx in range(NSM):
                    v.wait_ge(s_E, rep * NSM + idx + 1)
                    if rep >= 1:
                        # W tile reuse: all of the previous rep's output DMAs
                        # on the owning queue must be done (conservative --
                        # cross-DMA order within a queue isn't assumed)
                        qs = s_wact if idx < NSM // 2 else s_wsp
                        v.wait_ge(qs, 16 * (NSM // 2) * rep)
                    v.reciprocal(r_t[idx], sums_t[idx])
                    v.drain()  # r is a scalar operand of the next op
                    v.tensor_scalar_mul(W_t[idx], E_t[idx],
                                        r_t[idx]).then_inc(s_W, 1)

        @block.scalar
        def _(a):
            for rep in range(reps):
                for gg in range(GG):
                    _, size, _ = GLIST[gg]
                    a.wait_ge(s_T, rep * GG + gg + 1)
                    T = T_t[(rep * GG + gg) % 3]
                    a.activation(T[:, 0:size * LQ], T[:, 0:size * LQ],
                                 AF.Silu).then_inc(s_S, 1)
                for idx in range(NSM):
                    a.wait_ge(s_scm, rep * NSM + idx + 1)
                    if rep >= 1:
                        # E/sums tile reuse: previous rep's W-scale must be done
                        a.wait_ge(s_W, (rep - 1) * NSM + idx + 1)
                    a.activation(E_t[idx], sc_t[idx], AF.Exp,
                                 accum_out=sums_t[idx]).then_inc(s_E, 1)
                # early output tiles on ACT's HWDGE queue (waits are
                # already satisfied by the time the last exp retires)
                for idx in range(NSM // 2):
                    bh, ib = divmod(idx, NIB)
                    a.wait_ge(s_W, rep * NSM + idx + 1)
                    a.dma_start(out=w_d[bh, ib * 128:(ib + 1) * 128, :],
                                in_=W_t[idx]).then_inc(s_wact, 16)


        @block.tensor
        def _(t):
            t.wait_ge(s_qk0b, 16)  # a2 rides in its own tiny chunk
            for rep in range(reps):
                for gg in range(GG):
                    bh, size, off = GLIST[gg]
                    if rep == 0 and bh == 1 and off == 0:
                        t.wait_ge(s_qka[1], 16)
                    if rep >= 1 and off == 0:
                        # sc bank reuse: previous rep's exp must have consumed it
                        t.wait_ge(s_E, (rep - 1) * NSM + NIB * (bh + 1))
                    t.wait_ge(s_S, rep * GG + gg + 1)
                    T = T_t[(rep * GG + gg) % 3]
                    for pl in range(size):
                        p = off + pl
                        for ib in range(NIB):
                            ins = nc.tensor.matmul(
                                sc_t[bh * NIB + ib][:, 2 * p:2 * p + 2],
                                T[:, pl * LQ + ib * 128: pl * LQ + (ib + 1) * 128],
                                a2(bh),
                                start=True, stop=True)
                    ins.then_inc(s_pe, 1)

    return nc


def _shard(q, k, a, mask):
    qf = q.reshape(B * H, LQ, D)
    kf = k.reshape(B * H, LK, D)
    mf = mask.reshape(B * H, LQ, LK)
    af = np.ascontiguousarray(
        np.broadcast_to(a.reshape(1, H, D), (B, H, D))).reshape(B * H, D)
    in_maps = []
    for c in range(NCORES):
        sl = slice(NBH * c, NBH * (c + 1))
        qT = qf[sl].transpose(0, 2, 1)                                # [NBH,64,LQ]
        kp = kf[sl].reshape(NBH, NPAIR, 2, D).transpose(0, 2, 3, 1)   # [NBH,2,D,NPAIR]
        qka = np.zeros((NBH, 128, QKA), np.float32)
        qka[:, 0:64, 0:LQ] = qT
        qka[:, 64:128, 0:LQ] = qT
        qka[:, :, LQ:LQ + NPAIR] = kp.reshape(NBH, 128, NPAIR)
        for j in range(NBH):
            qka[j, 0:64, LQ + NPAIR] = af[NBH * c + j]
            qka[j, 64:128, LQ + NPAIR + 1] = af[NBH * c + j]
        masku8 = np.ascontiguousarray(mf[sl]).astype(np.uint8)
        in_maps.append(dict(qka=qka, masku8=masku8))
    return in_maps


def kernel(q, k, attention, mask):
    global _built
    q = np.asarray(q, np.float32)
    k = np.asarray(k, np.float32)
    a = np.asarray(attention, np.float32)
    mask = np.asarray(mask).astype(bool)

    in_maps = _shard(q, k, a, mask)
    if _built is None:
        _built = _build()
    res = run_bass_kernel_spmd(_built, in_maps, core_ids=list(range(NCORES)))
    w = np.stack([res.results[c]["w"] for c in range(NCORES)], axis=0)
    return w.reshape(B, H, LQ, LK).astype(np.float32)
